# revision 1
# baseline (speedup 1.0000x reference)
"""Trainium2 Bass kernel for nn_AttentiveTransformer (TabNet attentive transformer).

Computes, for full inputs (N=16384, NA=256, F=2048):
    x  = a @ W.T + b
    xn = batchnorm(x)  (training mode, batch stats over all N rows)
    m  = sparsemax_ascending_variant(xn * ps)
    new_ps = ps * (1.5 - m)

Key identities used:
 * The reference "sparsemax" sorts ascending; its k_z condition is monotone in
   the index, so k_z = D-1 always holds for this data regime and
   tau = (sum(z)+1)/(D-1), m = relu(z - tau). No sort.
 * BN stats from Gram partials: S1[f] = sum_r a_r.W_f, S2[f] = diag(W G W^T),
   both linear in per-core contributions -> one tiny AllReduce.
   var = S2/N - (S1/N)^2; the affine normalization is folded into the matmul:
   W' = W*s, bias t = bn_b - (S1/N)*s (b cancels).
 * All heavy matmuls run as bf16 hi/lo split passes (hi.hi + hi.lo + lo.hi),
   accumulating in fp32 PSUM: bf16 products are exact in fp32, so only the
   lo.lo term is dropped (~2^-18 relative) - fp32-grade accuracy at bf16
   speed (fp32 matmuls on trn2 lower to 2 hi/lo passes at ~2.6 cyc/col).

Sharding: data-parallel over rows, 2048 rows/core on 8 cores; a single 16KB
AllReduce merges the BN stats.
"""

import os
import sys
import numpy as np

for _p in ("/opt/trn_rl_repo",):
    if _p not in sys.path:
        sys.path.insert(0, _p)

KSTAGE = int(os.environ.get("KSTAGE", "9"))   # debug bisect: 9 = full kernel
KVAR = os.environ.get("KVAR", "")             # debug variant flags

N, NA, F = 16384, 256, 2048
NCORES = 8
NSH = N // NCORES            # 2048 rows per core
P = 128                      # partitions
RT = NSH // P                # 16 row-tiles per core
FCW = 512                    # feature chunk width (psum bank / max moving free)
FC = F // FCW                # 4 feature chunks
FP = F // P                  # 16 (cols of the [128,16] stats layout)
NAUG = NA + 1                # 257: a with ones column (colsum rides the Gram)
GAMMA = 1.5
BN_EPS = 1e-5
INV_D1 = 1.0 / (F - 1.0)     # 1/2047

_CACHE = {}


def _build_bass():
    import concourse.mybir as mybir
    import concourse.tile as tile
    from concourse import bacc
    from concourse.bass import ts

    fp32 = mybir.dt.float32
    bf16 = mybir.dt.bfloat16
    Alu = mybir.AluOpType
    Act = mybir.ActivationFunctionType

    nc = bacc.Bacc(
        "TRN2",
        target_bir_lowering=False,
        debug=False,
        enable_asserts=False,
        num_devices=NCORES,
    )

    # I/O (per core). a comes pre-split into bf16 hi/lo parts, in both
    # row-major (Gram, with a ones/zeros column appended) and transposed
    # (main matmul lhsT) layouts.
    ah_aug = nc.dram_tensor("ah_aug", [NSH, NAUG], bf16, kind="ExternalInput").ap()
    al2_aug = nc.dram_tensor("al2_aug", [NSH, NAUG], bf16, kind="ExternalInput").ap()
    ahT = nc.dram_tensor("ahT", [NA, NSH], bf16, kind="ExternalInput").ap()
    alT = nc.dram_tensor("alT", [NA, NSH], bf16, kind="ExternalInput").ap()
    wT = nc.dram_tensor("wT", [NA, F], fp32, kind="ExternalInput").ap()
    ps_in = nc.dram_tensor("ps_in", [NSH, F], fp32, kind="ExternalInput").ap()
    bnw16 = nc.dram_tensor("bnw16", [P, FP], fp32, kind="ExternalInput").ap()
    bnb16 = nc.dram_tensor("bnb16", [P, FP], fp32, kind="ExternalInput").ap()
    m_out = nc.dram_tensor("m_out", [NSH, F], fp32, kind="ExternalOutput").ap()
    nps_out = nc.dram_tensor("nps_out", [NSH, F], fp32, kind="ExternalOutput").ap()

    ps_t = ps_in.rearrange("(t p) f -> t p f", p=P)
    m_t = m_out.rearrange("(t p) f -> t p f", p=P)
    nps_t = nps_out.rearrange("(t p) f -> t p f", p=P)
    # a shard in [jumbo, p, sub, col] form: row = j*512 + s*128 + p
    ah_j = ah_aug.rearrange("(j s p) c -> j p s c", p=P, s=4)
    al_j = al2_aug.rearrange("(j s p) c -> j p s c", p=P, s=4)

    with tile.TileContext(nc) as tc:
        with tc.tile_pool(name="res", bufs=1) as res, \
             tc.tile_pool(name="dram", bufs=1, space="DRAM") as dram:
          if True:
            pro = tc.alloc_tile_pool(name="pro", bufs=1)

            # ---------------- constants ----------------
            ones_colb = pro.tile([P, 1], bf16)
            nc.vector.memset(ones_colb, 1.0)
            # preload the ACT table set (Sqrt + fillers) off the critical path
            warm = pro.tile([1, 1], fp32)
            nc.vector.memset(warm, 1.0)
            nc.scalar.activation(warm, warm, Act.Sqrt)

            # ---------------- phase 1: Gram partials (bf16 x3, ones col) ----
            # G_aug = a_aug^T a_aug approx ah^T ah_aug + ah^T al_aug + al^T ah_aug;
            # column NA of ah_aug is ones (al_aug's is zeros), so column NA of
            # G_aug is the colsum of a.
            g0h = pro.tile([P, NA], bf16)
            g0l = pro.tile([P, NA], bf16)
            g1h = pro.tile([P, NA], bf16)
            g1l = pro.tile([P, NA], bf16)
            sc0h = pro.tile([P, 1], bf16)
            sc0l = pro.tile([P, 1], bf16)
            sc1h = pro.tile([P, 1], bf16)
            sc1l = pro.tile([P, 1], bf16)
            with tc.tile_pool(name="pro1", bufs=1, space="PSUM") as pp1, \
                 tc.tile_pool(name="abig", bufs=2) as abigp:
                pg0 = pp1.tile([P, NAUG], fp32)
                pg1 = pp1.tile([P, NAUG], fp32)
                for j in range(4):
                    hch = abigp.tile([P, 4, NAUG], bf16, name="hch")
                    nc.sync.dma_start(hch, ah_j[j])
                    lch = abigp.tile([P, 4, NAUG], bf16, name="lch")
                    nc.sync.dma_start(lch, al_j[j])
                    for s in range(4):
                        first = j == 0 and s == 0
                        last = j == 3 and s == 3
                        ah_t = hch[:, s, :]
                        al_t = lch[:, s, :]
                        for half, pg in ((0, pg0), (1, pg1)):
                            hsl = ts(half, P)
                            nc.tensor.matmul(pg, ah_t[:, hsl], ah_t, start=first, stop=False)
                            nc.tensor.matmul(pg, ah_t[:, hsl], al_t, start=False, stop=last)
                # evict + split G and colsum into bf16 hi/lo
                for pg, gh, gl, sch, scl in ((pg0, g0h, g0l, sc0h, sc0l),
                                             (pg1, g1h, g1l, sc1h, sc1l)):
                    nc.vector.tensor_copy(gh, pg[:, 0:NA])
                    nc.vector.tensor_tensor(gl, pg[:, 0:NA], gh, Alu.subtract)
                    nc.scalar.copy(sch, pg[:, NA:NAUG])
                    nc.vector.tensor_tensor(scl, pg[:, NA:NAUG], sch, Alu.subtract)

            # ---------------- resident loads (after gram chunks hit the ring) ---
            ah0 = res.tile([P, NSH], bf16)
            nc.sync.dma_start(ah0, ahT[0:P, :])
            ah1 = res.tile([P, NSH], bf16)
            nc.sync.dma_start(ah1, ahT[P:NA, :])
            al0 = res.tile([P, NSH], bf16)
            nc.sync.dma_start(al0, alT[0:P, :])
            al1 = res.tile([P, NSH], bf16)
            nc.sync.dma_start(al1, alT[P:NA, :])
            bnw_c = pro.tile([P, FP], fp32)
            nc.sync.dma_start(bnw_c, bnw16)
            bnb_c = pro.tile([P, FP], fp32)
            nc.sync.dma_start(bnb_c, bnb16)

            # ---------------- phase 2: S1/S2 partials ----------------
            # raw-W bf16 splits (for H = G @ W^T and S1 = colsum^T @ W^T)
            wt0 = pro.tile([P, F], fp32)
            nc.sync.dma_start(wt0, wT[0:P, :])
            wt1 = pro.tile([P, F], fp32)
            nc.sync.dma_start(wt1, wT[P:NA, :])
            w0hr = pro.tile([P, F], bf16)
            nc.vector.tensor_copy(w0hr, wt0)
            w0lr = pro.tile([P, F], bf16)
            nc.vector.tensor_tensor(w0lr, wt0, w0hr, Alu.subtract)
            w1hr = pro.tile([P, F], bf16)
            nc.vector.tensor_copy(w1hr, wt1)
            w1lr = pro.tile([P, F], bf16)
            nc.vector.tensor_tensor(w1lr, wt1, w1hr, Alu.subtract)

            srow = pro.tile([1, 2 * F], fp32)   # cols 0:F = S1 partial, F:2F = S2
            with tc.tile_pool(name="pro2", bufs=1, space="PSUM") as pp2, \
                 tc.tile_pool(name="qtmp", bufs=2) as qtmp:
                for fc in range(FC):
                    fsl = ts(fc, FCW)
                    ph0 = pp2.tile([P, FCW], fp32, name="ph0", tag="ph0", bufs=2)
                    nc.tensor.matmul(ph0, g0h[:, 0:P], w0hr[:, fsl], start=True, stop=False)
                    nc.tensor.matmul(ph0, g1h[:, 0:P], w1hr[:, fsl], start=False, stop=False)
                    nc.tensor.matmul(ph0, g0l[:, 0:P], w0hr[:, fsl], start=False, stop=False)
                    nc.tensor.matmul(ph0, g1l[:, 0:P], w1hr[:, fsl], start=False, stop=False)
                    nc.tensor.matmul(ph0, g0h[:, 0:P], w0lr[:, fsl], start=False, stop=False)
                    nc.tensor.matmul(ph0, g1h[:, 0:P], w1lr[:, fsl], start=False, stop=True)
                    ph1 = pp2.tile([P, FCW], fp32, name="ph1", tag="ph1", bufs=2)
                    nc.tensor.matmul(ph1, g0h[:, P:NA], w0hr[:, fsl], start=True, stop=False)
                    nc.tensor.matmul(ph1, g1h[:, P:NA], w1hr[:, fsl], start=False, stop=False)
                    nc.tensor.matmul(ph1, g0l[:, P:NA], w0hr[:, fsl], start=False, stop=False)
                    nc.tensor.matmul(ph1, g1l[:, P:NA], w1hr[:, fsl], start=False, stop=False)
                    nc.tensor.matmul(ph1, g0h[:, P:NA], w0lr[:, fsl], start=False, stop=False)
                    nc.tensor.matmul(ph1, g1h[:, P:NA], w1lr[:, fsl], start=False, stop=True)
                    qf0 = qtmp.tile([P, FCW], fp32, name="qf0")
                    nc.vector.tensor_tensor(qf0, ph0, wt0[:, fsl], Alu.mult)
                    qf1 = qtmp.tile([P, FCW], fp32, name="qf1")
                    nc.vector.tensor_tensor(qf1, ph1, wt1[:, fsl], Alu.mult)
                    q0 = qtmp.tile([P, FCW], bf16, name="q0")
                    nc.scalar.copy(q0, qf0)
                    q1 = qtmp.tile([P, FCW], bf16, name="q1")
                    nc.scalar.copy(q1, qf1)
                    q0l = qtmp.tile([P, FCW], bf16, name="q0l")
                    nc.vector.tensor_tensor(q0l, qf0, q0, Alu.subtract)
                    q1l = qtmp.tile([P, FCW], bf16, name="q1l")
                    nc.vector.tensor_tensor(q1l, qf1, q1, Alu.subtract)
                    ps2 = pp2.tile([1, FCW], fp32, name="ps2", tag="ps2", bufs=2)
                    nc.tensor.matmul(ps2, ones_colb, q0, start=True, stop=False)
                    nc.tensor.matmul(ps2, ones_colb, q1, start=False, stop=False)
                    nc.tensor.matmul(ps2, ones_colb, q0l, start=False, stop=False)
                    nc.tensor.matmul(ps2, ones_colb, q1l, start=False, stop=True)
                    ps1 = pp2.tile([1, FCW], fp32, name="ps1", tag="ps1", bufs=2)
                    nc.tensor.matmul(ps1, sc0h, w0hr[:, fsl], start=True, stop=False)
                    nc.tensor.matmul(ps1, sc1h, w1hr[:, fsl], start=False, stop=False)
                    nc.tensor.matmul(ps1, sc0l, w0hr[:, fsl], start=False, stop=False)
                    nc.tensor.matmul(ps1, sc1l, w1hr[:, fsl], start=False, stop=True)
                    nc.scalar.copy(srow[0:1, fsl], ps1)
                    nc.scalar.copy(srow[0:1, ts(FC + fc, FCW)], ps2)

            # ---------------- phase 3: AllReduce of S1,S2 (16KB) ----------------
            cc_in = dram.tile([1, 2 * F], fp32)
            cc_out = dram.tile([1, 2 * F], fp32, addr_space="Shared")
            nc.sync.dma_start(cc_in, srow)
            nc.gpsimd.collective_compute(
                "AllReduce",
                Alu.add,
                replica_groups=[list(range(NCORES))],
                ins=[cc_in.opt()],
                outs=[cc_out.opt()],
            )
            cc_r = cc_out.rearrange("o (two p c) -> two (o p) c", two=2, p=P)

            # ---------------- phase 4: stats math in [128,16] layout ----------------
            s_row = pro.tile([1, F], fp32)
            t_row = pro.tile([1, F], fp32)
            sh_row = pro.tile([1, F], bf16)
            sl_row = pro.tile([1, F], bf16)
            ttl2 = res.tile([2, F], bf16)       # row 0: t hi, row 1: t lo
            ones2 = res.tile([2, P], bf16)
            nc.vector.memset(ones2, 1.0)
            with tc.tile_pool(name="smath", bufs=1) as sm:
                st1 = sm.tile([P, FP], fp32)
                nc.sync.dma_start(st1, cc_r[0])
                st2 = sm.tile([P, FP], fp32)
                nc.sync.dma_start(st2, cc_r[1])
                sq = sm.tile([P, FP], fp32)
                nc.vector.tensor_tensor(sq, st1, st1, Alu.mult)
                # vv = S2 - S1^2/N + N*eps  (= N*(var+eps))
                vv = sm.tile([P, FP], fp32)
                nc.vector.scalar_tensor_tensor(vv, sq, -1.0 / N, st2, Alu.mult, Alu.add)
                nc.vector.tensor_scalar_add(vv, vv, float(N * BN_EPS))
                rr = sm.tile([P, FP], fp32)
                nc.scalar.activation(rr, vv, Act.Sqrt)
                y = sm.tile([P, FP], fp32)
                nc.vector.reciprocal(y, rr)
                # two Newton iterations for 1/sqrt(vv) (ScalarE Sqrt is low-precision)
                for _ in range(2):
                    yy = sm.tile([P, FP], fp32, name="yy", tag="yy", bufs=2)
                    nc.vector.tensor_tensor(yy, y, y, Alu.mult)
                    vyy = sm.tile([P, FP], fp32, name="vyy", tag="vyy", bufs=2)
                    nc.vector.tensor_tensor(vyy, vv, yy, Alu.mult)
                    w = sm.tile([P, FP], fp32, name="w", tag="w", bufs=2)
                    nc.vector.tensor_scalar(w, vyy, -0.5, 1.5, Alu.mult, Alu.add)
                    y2 = sm.tile([P, FP], fp32, name="y2", tag="y2", bufs=2)
                    nc.vector.tensor_tensor(y2, y, w, Alu.mult)
                    y = y2
                # s = sqrt(N) * y * bn_w.  The matmul uses W' = W*s with NO +b
                # term, and mu = S1/N + b, so the folded bias is
                # t = bn_b + (b - mu)*s = bn_b - (S1/N)*s  (b cancels).
                s_c = sm.tile([P, FP], fp32)
                nc.vector.scalar_tensor_tensor(s_c, y, float(np.sqrt(N)), bnw_c, Alu.mult, Alu.mult)
                tm = sm.tile([P, FP], fp32)
                nc.vector.scalar_tensor_tensor(tm, st1, -1.0 / N, s_c, Alu.mult, Alu.mult)
                t_c = sm.tile([P, FP], fp32)
                nc.vector.tensor_tensor(t_c, tm, bnb_c, Alu.add)
                # bf16 hi/lo splits in the 128-lane layout (cheap), then
                # direct SBUF->SBUF DMAs gather them into single-row tiles
                sh_c = sm.tile([P, FP], bf16)
                nc.vector.tensor_copy(sh_c, s_c)
                sl_c = sm.tile([P, FP], bf16)
                nc.vector.tensor_tensor(sl_c, s_c, sh_c, Alu.subtract)
                th_c = sm.tile([P, FP], bf16)
                nc.vector.tensor_copy(th_c, t_c)
                tl_c = sm.tile([P, FP], bf16)
                nc.vector.tensor_tensor(tl_c, t_c, th_c, Alu.subtract)
                nc.sync.dma_start(sh_row, sh_c)
                nc.sync.dma_start(sl_row, sl_c)
                nc.sync.dma_start(ttl2[0:1, :], th_c)
                nc.sync.dma_start(ttl2[1:2, :], tl_c)
                if KSTAGE < 6:
                    nc.sync.dma_start(s_row, s_c)
                    nc.sync.dma_start(t_row, t_c)

            # ---------------- phase 5: fold scale into W^T; bf16 splits -----
            w0h = res.tile([P, F], bf16)
            w0l = res.tile([P, F], bf16)
            w1h = res.tile([P, F], bf16)
            w1l = res.tile([P, F], bf16)
            ones_rowb = pro.tile([1, P], bf16)
            nc.vector.memset(ones_rowb, 1.0)
            with tc.tile_pool(name="pro3", bufs=2, space="PSUM") as pp3, \
                 tc.tile_pool(name="wsc", bufs=2) as wsc:
                for fc in range(FC):
                    fsl = ts(fc, FCW)
                    pb = pp3.tile([P, FCW], fp32, name="pb")
                    nc.tensor.matmul(pb, ones_rowb, sh_row[:, fsl], start=True, stop=False)
                    nc.tensor.matmul(pb, ones_rowb, sl_row[:, fsl], start=False, stop=True)
                    w0s = wsc.tile([P, FCW], fp32, name="w0s")
                    nc.vector.tensor_tensor(w0s, wt0[:, fsl], pb, Alu.mult)
                    w1s = wsc.tile([P, FCW], fp32, name="w1s")
                    nc.vector.tensor_tensor(w1s, wt1[:, fsl], pb, Alu.mult)
                    nc.vector.tensor_copy(w0h[:, fsl], w0s)
                    nc.vector.tensor_tensor(w0l[:, fsl], w0s, w0h[:, fsl], Alu.subtract)
                    nc.vector.tensor_copy(w1h[:, fsl], w1s)
                    nc.vector.tensor_tensor(w1l[:, fsl], w1s, w1h[:, fsl], Alu.subtract)

            if KSTAGE < 6:
                nc.sync.dma_start(m_out.rearrange("(t p) f -> t p f", p=1)[0], s_row)
                nc.sync.dma_start(nps_out.rearrange("(t p) f -> t p f", p=1)[0], t_row)
            pro.release()

            # ---------------- main loop over 16 row-tiles ----------------
            nrt_loop = (RT if KSTAGE >= 7 else 1) if KSTAGE >= 6 else 0
            with tc.tile_pool(name="mx", bufs=8, space="PSUM") as mxp, \
                 tc.tile_pool(name="psb", bufs=5) as psb, \
                 tc.tile_pool(name="zb", bufs=4) as zb, \
                 tc.tile_pool(name="mb", bufs=3) as mb, \
                 tc.tile_pool(name="qb", bufs=3) as qb, \
                 tc.tile_pool(name="nb", bufs=3) as nb, \
                 tc.tile_pool(name="rsb", bufs=4) as rsb:
                for rt in range(nrt_loop):
                    rsl = ts(rt, P)
                    pst = psb.tile([P, F], fp32, name="pst")
                    nc.sync.dma_start(pst, ps_t[rt])
                    zt = zb.tile([P, F], fp32, name="zt")
                    # pass-type-major: each lhsT is loaded once per row-tile and
                    # streams all 4 feature chunks (LDWEIGHTS dedupe-friendly)
                    px = mxp.tile([P, F], fp32, name="px", tag="px", bufs=2)
                    ptypes = [(ah0[:, rsl], w0h), (ah1[:, rsl], w1h),
                              (ah0[:, rsl], w0l), (ah1[:, rsl], w1l),
                              (al0[:, rsl], w0h), (al1[:, rsl], w1h),
                              (ones2, ttl2)]
                    for pi, (lhsT, rhs) in enumerate(ptypes):
                        for fc in range(FC):
                            nc.tensor.matmul(px[:, ts(fc, FCW)], lhsT, rhs[:, ts(fc, FCW)],
                                             start=(pi == 0), stop=(pi == len(ptypes) - 1))
                    # z' = -xn * ps over the whole row-tile; rs = rowsum(z')
                    rs = rsb.tile([P, 1], fp32, name="rs")
                    nc.vector.scalar_tensor_tensor(
                        zt, px, -1.0, pst, Alu.mult, Alu.mult, accum_out=rs,
                    )
                    # rs = -sum(z); tau = (1-rs... ) see: tau=(sum(z)+1)/2047=(1-rs)/2047
                    ntau = rsb.tile([P, 1], fp32, name="ntau")      # -tau
                    nc.vector.tensor_scalar(ntau, rs, INV_D1, -INV_D1, Alu.mult, Alu.add)
                    ctau = rsb.tile([P, 1], fp32, name="ctau")      # tau + GAMMA
                    nc.vector.tensor_scalar(ctau, rs, -INV_D1, INV_D1 + GAMMA, Alu.mult, Alu.add)
                    # m = relu(z - tau) = relu(-z' + ntau)
                    mt = mb.tile([P, F], fp32, name="mt")
                    nc.scalar.activation(mt, zt, Act.Relu, bias=ntau, scale=-1.0)
                    nc.sync.dma_start(m_t[rt], mt)
                    # GAMMA - m = min(z' + (tau+GAMMA), GAMMA)
                    ut = qb.tile([P, F], fp32, name="ut")
                    nc.vector.tensor_scalar(ut, zt, ctau, GAMMA, Alu.add, Alu.min)
                    nt = nb.tile([P, F], fp32, name="nt")
                    if "npsv" in KVAR or rt >= nrt_loop - 2:
                        nc.vector.tensor_tensor(nt, ut, pst, Alu.mult)
                    else:
                        nc.gpsimd.tensor_tensor(nt, ut, pst, Alu.mult)
                    nc.scalar.dma_start(nps_t[rt], nt)

    nc.compile()
    return nc


def _get_nc():
    if "nc" not in _CACHE:
        _CACHE["nc"] = _build_bass()
    return _CACHE["nc"]


def _bf16_split(x):
    import ml_dtypes
    hi = x.astype(ml_dtypes.bfloat16)
    lo = (x - hi.astype(np.float32)).astype(ml_dtypes.bfloat16)
    return hi, lo


def _make_in_maps(a, ps, W, b, bn_w, bn_b):
    a = np.ascontiguousarray(a, dtype=np.float32)
    ps = np.ascontiguousarray(ps, dtype=np.float32)
    wT_np = np.ascontiguousarray(W.astype(np.float32).T)
    bnw16 = np.ascontiguousarray(bn_w.astype(np.float32).reshape(P, FP))
    bnb16 = np.ascontiguousarray(bn_b.astype(np.float32).reshape(P, FP))
    ah, al = _bf16_split(a)
    in_maps = []
    for c in range(NCORES):
        rows = slice(c * NSH, (c + 1) * NSH)
        ah_c = ah[rows]
        al_c = al[rows]
        ah_aug = np.concatenate([ah_c, np.ones((NSH, 1), ah.dtype)], axis=1)
        al2_aug = np.concatenate([al_c * np.asarray(2.0, al.dtype),
                                  np.zeros((NSH, 1), al.dtype)], axis=1)
        in_maps.append({
            "ah_aug": np.ascontiguousarray(ah_aug),
            "al2_aug": np.ascontiguousarray(al2_aug),
            "ahT": np.ascontiguousarray(ah_c.T),
            "alT": np.ascontiguousarray(al_c.T),
            "wT": wT_np,
            "ps_in": np.ascontiguousarray(ps[rows]),
            "bnw16": bnw16,
            "bnb16": bnb16,
        })
    return in_maps


def _maybe_patch_ldwopt():
    """Optionally flip walrus's --enable-ldw-opt (default false in bass_utils)."""
    if os.environ.get("BASS_LDW_OPT") != "1":
        return
    from concourse import bass_utils as bu
    if getattr(bu, "_ldwopt_patched", False):
        return
    orig = bu.run_command

    def patched(argv, **kw):
        argv = [x.replace("--enable-ldw-opt=false", "--enable-ldw-opt=true")
                if isinstance(x, str) else x for x in argv]
        return orig(argv, **kw)

    bu.run_command = patched
    bu._ldwopt_patched = True


def run(a, ps, W, b, bn_w, bn_b, trace=False, **kw):
    """Run the kernel on the 8 NeuronCores; returns ((m, new_ps), BassKernelResults)."""
    from concourse import bass_utils

    _maybe_patch_ldwopt()

    nc = _get_nc()
    in_maps = _make_in_maps(a, ps, W, b, bn_w, bn_b)
    res = bass_utils.run_bass_kernel_spmd(
        nc, in_maps, core_ids=list(range(NCORES)), trace=trace, **kw,
    )
    m = np.concatenate([r["m_out"] for r in res.results], axis=0)
    nps = np.concatenate([r["nps_out"] for r in res.results], axis=0)
    return (m, nps), res


def kernel(a, ps, W, b, bn_w, bn_b):
    (m, nps), _ = run(a, ps, W, b, bn_w, bn_b, trace=False)
    return m, nps


if __name__ == "__main__":
    rng = np.random.default_rng(0)
    a = rng.standard_normal((N, NA), dtype=np.float32)
    ps = rng.random((N, F), dtype=np.float32)
    lim = 1.0 / np.sqrt(NA)
    W = rng.uniform(-lim, lim, (F, NA)).astype(np.float32)
    b = rng.uniform(-lim, lim, (F,)).astype(np.float32)
    bn_w = np.ones((F,), np.float32)
    bn_b = np.zeros((F,), np.float32)
    (m, nps), res = run(a, ps, W, b, bn_w, bn_b)
    print("m", m.shape, m.dtype, "nps", nps.shape)
    print("exec_time_ns:", res.exec_time_ns)

